# revision 6
# baseline (speedup 1.0000x reference)
"""Trainium2 Bass kernel for nn_AdditiveAttention (Bahdanau attention).

Reference computation (B=16, Q=128, K=128, D=512, H=512):
    q = queries @ Wq                     [B,Q,H]
    k = keys @ Wk                        [B,K,H]
    scores[b,q,k] = sum_h wv[h] * tanh(q[b,q,h] + k[b,k,h])
    attn = softmax over valid keys (k < valid_lens[b])
    out = attn @ values                  [B,Q,D]

Strategy (8 NeuronCores, data-parallel over batch):
  - 2 batches per core, in two "slots".  Slot sizes V0/V1 = max valid_len
    over the cores' slot-0/slot-1 batches; batches are assigned to slots
    sorted by valid_len so V0+V1 is minimal.  Columns >= the batch's own
    valid_len are masked with a -1e9 additive mask (softmax -> 0).
  - On-device per slot: project queriesT/keysT transposed ([h=partitions]),
    per key-column k: pre[h,q] = q_projT + k_col (DVE tensor_scalar, 4x
    mode), tanh on ScalarE in big batched instructions, then the wv
    reduction on TensorE (lhsT = tanh tile [128h,128q], rhs = wv chunk
    [128h,1] -> scores column per h-chunk into 4 per-chunk PSUM banks),
    bank-sum + mask on DVE, exp with fused accumulated sum on ScalarE,
    reciprocal on DVE, transpose of the exp matrix on TensorE, and a final
    attn @ values matmul, scaled by 1/z.
All heavy compute is bf16 on PE/DVE with fp32 PSUM accumulation; tanh/exp
run in fp32 internally on ScalarE.  Masked key columns are skipped
entirely (only V0+V1 of 256 columns are computed).
"""

import os
import sys
import types
import math
import numpy as np
import ml_dtypes

# ---------------------------------------------------------------------------
# axon NTFF profile hook (lets trace=True / BASS_TRACE=1 work in this image)
# ---------------------------------------------------------------------------
def _install_axon_hooks():
    if "antenv.axon_hooks" in sys.modules:
        return
    try:
        import trn_agent_boot.trn_boot as _tb

        _hooks = types.ModuleType("antenv.axon_hooks")
        _hook = _tb._ntff_profile_via_ctypes("/opt/axon/libaxon_pjrt.so")
        _hooks.get_axon_ntff_profile_hook = lambda: _hook
        _hooks.set_axon_ntff_profile_hook = lambda h: None
        sys.modules["antenv.axon_hooks"] = _hooks
    except Exception:
        pass


_install_axon_hooks()

import concourse.bass as bass
import concourse.bacc as bacc
import concourse.mybir as mybir
import concourse.tile as tile
import concourse.bass_utils as bass_utils
from concourse.bass_utils import run_bass_kernel_spmd
from concourse.masks import make_identity

# Avoid S3 artifact-upload attempts in the trace path.
bass_utils.upload_artifacts = lambda tmpdir: tmpdir

F32 = mybir.dt.float32
BF16 = mybir.dt.bfloat16
BF16_NP = ml_dtypes.bfloat16

B, Q, K, D, H = 16, 128, 128, 512, 512
NCORES = 8
KT = 32  # key-columns per tanh group
NEG = -1e9

_NC_CACHE: dict = {}


def _build_nc(V0: int, V1: int):
    """Build + finalize the single-core SPMD program for slot sizes V0, V1."""
    nc = bacc.Bacc(None, target_bir_lowering=False, debug=False)

    qT = nc.declare_dram_parameter("qT", [2, D, Q], BF16, isOutput=False)
    kT = nc.declare_dram_parameter("kT", [2, D, K], BF16, isOutput=False)
    vals = nc.declare_dram_parameter("vals", [2, K, D], BF16, isOutput=False)
    wq_d = nc.declare_dram_parameter("wq", [D, H], BF16, isOutput=False)
    wk_d = nc.declare_dram_parameter("wk", [D, H], BF16, isOutput=False)
    wv_d = nc.declare_dram_parameter("wv4", [128, 4], BF16, isOutput=False)
    mask_d = nc.declare_dram_parameter("mask", [2, 128, K], F32, isOutput=False)
    out_d = nc.declare_dram_parameter("out", [2, Q, D], F32, isOutput=True)

    Vs = [V0, V1]
    Tanh = mybir.ActivationFunctionType.Tanh
    Exp = mybir.ActivationFunctionType.Exp

    with tile.TileContext(nc) as tc:
        with (
            tc.tile_pool(name="const", bufs=1) as constp,
            tc.tile_pool(name="io", bufs=1) as iop,
            tc.tile_pool(name="proj", bufs=1) as projp,
            tc.tile_pool(name="stage", bufs=2) as stagep,
            tc.tile_pool(name="sm", bufs=2) as smp,
            tc.tile_pool(name="ps_proj", bufs=2, space="PSUM") as ps_proj,
            tc.tile_pool(name="ps_sc", bufs=4, space="PSUM") as ps_sc,
            tc.tile_pool(name="ps_misc", bufs=1, space="PSUM") as ps_misc,
        ):
            # ---- constants & inputs -------------------------------------
            wq_sb = constp.tile([128, 4, H], BF16, tag="wq")
            nc.sync.dma_start(wq_sb[:], wq_d[:].rearrange("(c p) h -> p c h", p=128))
            wk_sb = constp.tile([128, 4, H], BF16, tag="wk")
            nc.sync.dma_start(wk_sb[:], wk_d[:].rearrange("(c p) h -> p c h", p=128))
            wv_sb = constp.tile([128, 4], BF16, tag="wv")
            nc.sync.dma_start(wv_sb[:], wv_d[:])
            ident = constp.tile([128, 128], BF16, tag="ident")
            make_identity(nc, ident[:])

            qt_sb = iop.tile([128, 2, 4, Q], BF16, tag="qt")
            nc.sync.dma_start(qt_sb[:], qT[:].rearrange("s (c p) q -> p s c q", p=128))
            kt_sb = iop.tile([128, 2, 4, K], BF16, tag="kt")
            nc.sync.dma_start(kt_sb[:], kT[:].rearrange("s (c p) k -> p s c k", p=128))
            vals_sb = iop.tile([128, 2, D], BF16, tag="vals")
            nc.sync.dma_start(vals_sb[:], vals[:].rearrange("s k d -> k s d"))
            mask_sb = iop.tile([128, 2, K], F32, tag="mask")
            nc.sync.dma_start(mask_sb[:], mask_d[:].rearrange("s p k -> p s k"))

            # ---- projections: projT[h,q] = sum_d W[d,h] * xT[d,q] -------
            qproj = projp.tile([128, 2, 4, Q], BF16, tag="qproj")
            kproj = projp.tile([128, 2, 4, K], F32, tag="kproj")
            for s in (0, 1):
                V = Vs[s]
                for hc in range(4):
                    pq = ps_proj.tile([128, 128], F32, tag="pp")
                    for dc in range(4):
                        nc.tensor.matmul(
                            pq[:],
                            wq_sb[:, dc, hc * 128 : (hc + 1) * 128],
                            qt_sb[:, s, dc, :],
                            start=(dc == 0),
                            stop=(dc == 3),
                        )
                    nc.scalar.copy(qproj[:, s, hc, :], pq[:])
                    pk = ps_proj.tile([128, 128], F32, tag="pp")
                    for dc in range(4):
                        nc.tensor.matmul(
                            pk[:, :V],
                            wk_sb[:, dc, hc * 128 : (hc + 1) * 128],
                            kt_sb[:, s, dc, :V],
                            start=(dc == 0),
                            stop=(dc == 3),
                        )
                    nc.vector.tensor_copy(kproj[:, s, hc, :V], pk[:, :V])

            # persistent softmax state
            e_sb = projp.tile([128, 2, K], BF16, tag="e")
            nc.vector.memset(e_sb[:], 0.0)
            rz = projp.tile([128, 2], F32, tag="rz")

            # ---- main loop ----------------------------------------------
            for s in (0, 1):
                V = Vs[s]
                ngroups = math.ceil(V / KT)
                # 4 per-h-chunk score banks, each column written exactly once
                psc = [
                    ps_sc.tile([128, K], F32, tag="psc", name=f"psc{s}_{i}")
                    for i in range(4)
                ]
                for g in range(ngroups):
                    k0 = g * KT
                    Kg = min(KT, V - k0)
                    pre = stagep.tile([128, 4, KT, Q], BF16, tag="pre")
                    tnh = stagep.tile([128, 4, KT, Q], BF16, tag="tnh")
                    for hc in range(4):
                        for kl in range(Kg):
                            nc.vector.tensor_scalar_add(
                                pre[:, hc, kl, :],
                                qproj[:, s, hc, :],
                                kproj[:, s, hc, k0 + kl : k0 + kl + 1],
                            )
                    nc.scalar.activation(
                        tnh[:, :, :Kg, :], pre[:, :, :Kg, :], Tanh
                    )
                    for kl in range(Kg):
                        for hc in range(4):
                            nc.tensor.matmul(
                                psc[hc][:, k0 + kl : k0 + kl + 1],
                                tnh[:, hc, kl, :],
                                wv_sb[:, hc : hc + 1],
                                start=True,
                                stop=True,
                            )
                # ---- softmax over the V columns -------------------------
                msc = smp.tile([128, K], F32, tag="msc")
                nc.vector.tensor_add(msc[:, :V], psc[0][:, :V], mask_sb[:, s, :V])
                nc.vector.tensor_add(msc[:, :V], msc[:, :V], psc[1][:, :V])
                nc.vector.tensor_add(msc[:, :V], msc[:, :V], psc[2][:, :V])
                nc.vector.tensor_add(msc[:, :V], msc[:, :V], psc[3][:, :V])
                z = smp.tile([128, 1], F32, tag="z")
                nc.scalar.activation(
                    e_sb[:, s, :V], msc[:, :V], Exp, accum_out=z[:]
                )
                nc.vector.reciprocal(rz[:, s : s + 1], z[:])
                # ---- attn @ values --------------------------------------
                pt = ps_misc.tile([128, 128], BF16, tag="pt")
                nc.tensor.transpose(pt[:], e_sb[:, s, :], ident[:])
                eT = smp.tile([128, 128], BF16, tag="eT")
                nc.vector.tensor_copy(eT[:], pt[:])
                po = ps_misc.tile([128, D], F32, tag="po")
                nc.tensor.matmul(
                    po[:], eT[:V, :], vals_sb[:V, s, :], start=True, stop=True
                )
                o_sb = smp.tile([128, D], F32, tag="o")
                nc.vector.tensor_scalar_mul(o_sb[:], po[:], rz[:, s : s + 1])
                nc.sync.dma_start(out_d[s], o_sb[:])

    nc.finalize()
    return nc


def kernel(queries, keys, values, valid_lens, Wq, Wk, wv):
    queries = np.asarray(queries, dtype=np.float32)
    keys = np.asarray(keys, dtype=np.float32)
    values = np.asarray(values, dtype=np.float32)
    valid_lens = np.asarray(valid_lens, dtype=np.int32)
    Wq = np.asarray(Wq, dtype=np.float32)
    Wk = np.asarray(Wk, dtype=np.float32)
    wv = np.asarray(wv, dtype=np.float32)

    # ---- slot assignment: minimize V0 + V1 ------------------------------
    order = np.argsort(-valid_lens, kind="stable")
    slot0 = order[:NCORES]
    slot1 = order[NCORES:][::-1]  # pair largest with smallest
    V0 = int(valid_lens[slot0].max())
    V1 = int(valid_lens[slot1].max())

    key_ = (V0, V1)
    if key_ not in _NC_CACHE:
        _NC_CACHE[key_] = _build_nc(V0, V1)
    nc = _NC_CACHE[key_]

    # ---- host-side shard prep -------------------------------------------
    wq_bf = Wq.astype(BF16_NP)
    wk_bf = Wk.astype(BF16_NP)
    wv4 = np.ascontiguousarray(wv.reshape(4, 128).T).astype(BF16_NP)  # [128,4]

    in_maps = []
    for c in range(NCORES):
        bs = (int(slot0[c]), int(slot1[c]))
        qT = np.stack(
            [np.ascontiguousarray(queries[b].T) for b in bs]
        ).astype(BF16_NP)  # [2, D, Q]
        kTm = np.zeros((2, D, K), dtype=BF16_NP)
        valsm = np.zeros((2, K, D), dtype=BF16_NP)
        maskm = np.zeros((2, 128, K), dtype=np.float32)
        for s, b in enumerate(bs):
            v = int(valid_lens[b])
            kTm[s, :, :v] = keys[b, :v].T.astype(BF16_NP)
            valsm[s, :v] = values[b, :v].astype(BF16_NP)
            maskm[s, :, v:] = NEG
        in_maps.append(
            {
                "qT": qT,
                "kT": kTm,
                "vals": valsm,
                "wq": wq_bf,
                "wk": wk_bf,
                "wv4": wv4,
                "mask": maskm,
            }
        )

    res = run_bass_kernel_spmd(nc, in_maps, list(range(NCORES)))
    global LAST_RESULT
    LAST_RESULT = res

    out = np.empty((B, Q, D), dtype=np.float32)
    for c in range(NCORES):
        o = np.asarray(res.results[c]["out"], dtype=np.float32)
        out[int(slot0[c])] = o[0]
        out[int(slot1[c])] = o[1]
    return out


# revision 8
# speedup vs baseline: 1.0185x; 1.0185x over previous
"""Trainium2 Bass kernel for nn_AdditiveAttention (Bahdanau attention).

Reference computation (B=16, Q=128, K=128, D=512, H=512):
    q = queries @ Wq                     [B,Q,H]
    k = keys @ Wk                        [B,K,H]
    scores[b,q,k] = sum_h wv[h] * tanh(q[b,q,h] + k[b,k,h])
    attn = softmax over valid keys (k < valid_lens[b])
    out = attn @ values                  [B,Q,D]

Strategy (8 NeuronCores, data-parallel over batch):
  - 2 batches per core, in two "slots".  Slot sizes V0/V1 = max valid_len
    over the cores' slot-0/slot-1 batches; batches are assigned to slots
    sorted by valid_len so V0+V1 is minimal.  Columns >= the batch's own
    valid_len are masked with a -1e9 additive mask (softmax -> 0).
  - On-device per slot: project queriesT/keysT transposed ([h=partitions]),
    per key-column k: pre[h,q] = q_projT + k_col (DVE tensor_scalar, 4x
    mode), tanh on ScalarE in big batched instructions, then the wv
    reduction on TensorE (lhsT = tanh tile [128h,128q], rhs = wv chunk
    [128h,1] -> scores column per h-chunk into 4 per-chunk PSUM banks),
    bank-sum + mask on DVE, exp with fused accumulated sum on ScalarE,
    reciprocal on DVE, transpose of the exp matrix on TensorE, and a final
    attn @ values matmul, scaled by 1/z.
All heavy compute is bf16 on PE/DVE with fp32 PSUM accumulation; tanh/exp
run in fp32 internally on ScalarE.  Masked key columns are skipped
entirely (only V0+V1 of 256 columns are computed).
"""

import os
import sys
import types
import math
import numpy as np
import ml_dtypes

# ---------------------------------------------------------------------------
# axon NTFF profile hook (lets trace=True / BASS_TRACE=1 work in this image)
# ---------------------------------------------------------------------------
def _install_axon_hooks():
    if "antenv.axon_hooks" in sys.modules:
        return
    try:
        import trn_agent_boot.trn_boot as _tb

        _hooks = types.ModuleType("antenv.axon_hooks")
        _hook = _tb._ntff_profile_via_ctypes("/opt/axon/libaxon_pjrt.so")
        _hooks.get_axon_ntff_profile_hook = lambda: _hook
        _hooks.set_axon_ntff_profile_hook = lambda h: None
        sys.modules["antenv.axon_hooks"] = _hooks
    except Exception:
        pass


_install_axon_hooks()

import concourse.bass as bass
import concourse.bacc as bacc
import concourse.mybir as mybir
import concourse.tile as tile
import concourse.bass_utils as bass_utils
from concourse.bass_utils import run_bass_kernel_spmd
from concourse.masks import make_identity

# Avoid S3 artifact-upload attempts in the trace path.
bass_utils.upload_artifacts = lambda tmpdir: tmpdir

F32 = mybir.dt.float32
BF16 = mybir.dt.bfloat16
BF16_NP = ml_dtypes.bfloat16

B, Q, K, D, H = 16, 128, 128, 512, 512
NCORES = 8
KT = 32  # key-columns per tanh group
NEG = -1e9

_NC_CACHE: dict = {}


def _build_nc(V0: int, V1: int):
    """Build + finalize the single-core SPMD program for slot sizes V0, V1."""
    nc = bacc.Bacc(None, target_bir_lowering=False, debug=False)

    qT = nc.declare_dram_parameter("qT", [2, D, Q], BF16, isOutput=False)
    kT = nc.declare_dram_parameter("kT", [2, D, K], BF16, isOutput=False)
    vals = nc.declare_dram_parameter("vals", [2, K, D], BF16, isOutput=False)
    wq_d = nc.declare_dram_parameter("wq", [D, H], BF16, isOutput=False)
    wk_d = nc.declare_dram_parameter("wk", [D, H], BF16, isOutput=False)
    wv_d = nc.declare_dram_parameter("wv4", [128, 4], BF16, isOutput=False)
    mask_d = nc.declare_dram_parameter("mask", [2, 128, K], F32, isOutput=False)
    out_d = nc.declare_dram_parameter("out", [2, Q, D], F32, isOutput=True)

    Vs = [V0, V1]
    Tanh = mybir.ActivationFunctionType.Tanh
    Exp = mybir.ActivationFunctionType.Exp

    with tile.TileContext(nc) as tc:
        with (
            tc.tile_pool(name="const", bufs=1) as constp,
            tc.tile_pool(name="io", bufs=1) as iop,
            tc.tile_pool(name="proj", bufs=1) as projp,
            tc.tile_pool(name="stage", bufs=2) as stagep,
            tc.tile_pool(name="sm", bufs=2) as smp,
            tc.tile_pool(name="ps_proj", bufs=2, space="PSUM") as ps_proj,
            tc.tile_pool(name="ps_sc", bufs=4, space="PSUM") as ps_sc,
            tc.tile_pool(name="ps_misc", bufs=1, space="PSUM") as ps_misc,
        ):
            # ---- constants & inputs -------------------------------------
            wq_sb = constp.tile([128, 4, H], BF16, tag="wq")
            nc.sync.dma_start(wq_sb[:], wq_d[:].rearrange("(c p) h -> p c h", p=128))
            wk_sb = constp.tile([128, 4, H], BF16, tag="wk")
            nc.sync.dma_start(wk_sb[:], wk_d[:].rearrange("(c p) h -> p c h", p=128))
            wv_sb = constp.tile([128, 4], BF16, tag="wv")
            nc.sync.dma_start(wv_sb[:], wv_d[:])
            ident = constp.tile([128, 128], BF16, tag="ident")
            make_identity(nc, ident[:])

            qt_sb = iop.tile([128, 2, 4, Q], BF16, tag="qt")
            nc.sync.dma_start(qt_sb[:], qT[:].rearrange("s (c p) q -> p s c q", p=128))
            kt_sb = iop.tile([128, 2, 4, K], BF16, tag="kt")
            nc.sync.dma_start(kt_sb[:], kT[:].rearrange("s (c p) k -> p s c k", p=128))
            vals_sb = iop.tile([128, 2, D], BF16, tag="vals")
            nc.sync.dma_start(vals_sb[:], vals[:].rearrange("s k d -> k s d"))
            mask_sb = iop.tile([128, 2, K], F32, tag="mask")
            nc.sync.dma_start(mask_sb[:], mask_d[:].rearrange("s p k -> p s k"))

            # ---- projections: projT[h,q] = sum_d W[d,h] * xT[d,q] -------
            qproj = projp.tile([128, 2, 4, Q], BF16, tag="qproj")
            kproj = projp.tile([128, 2, 4, K], BF16, tag="kproj")
            for s in (0, 1):
                V = Vs[s]
                for hc in range(4):
                    pq = ps_proj.tile([128, 128], F32, tag="pp")
                    for dc in range(4):
                        nc.tensor.matmul(
                            pq[:],
                            wq_sb[:, dc, hc * 128 : (hc + 1) * 128],
                            qt_sb[:, s, dc, :],
                            start=(dc == 0),
                            stop=(dc == 3),
                        )
                    nc.scalar.copy(qproj[:, s, hc, :], pq[:])
                    pk = ps_proj.tile([128, 128], F32, tag="pp")
                    for dc in range(4):
                        nc.tensor.matmul(
                            pk[:, :V],
                            wk_sb[:, dc, hc * 128 : (hc + 1) * 128],
                            kt_sb[:, s, dc, :V],
                            start=(dc == 0),
                            stop=(dc == 3),
                        )
                    nc.vector.tensor_copy(kproj[:, s, hc, :V], pk[:, :V])

            # persistent softmax state
            e_sb = projp.tile([128, 2, K], BF16, tag="e")
            nc.vector.memset(e_sb[:], 0.0)
            rz = projp.tile([128, 2], F32, tag="rz")

            # ---- main loop ----------------------------------------------
            for s in (0, 1):
                V = Vs[s]
                ngroups = math.ceil(V / KT)
                # 4 per-h-chunk score banks, each column written exactly once
                psc = [
                    ps_sc.tile([128, K], F32, tag="psc", name=f"psc{s}_{i}")
                    for i in range(4)
                ]
                for g in range(ngroups):
                    k0 = g * KT
                    Kg = min(KT, V - k0)
                    pre = stagep.tile([128, 4, KT, Q], BF16, tag="pre")
                    tnh = stagep.tile([128, 4, KT, Q], BF16, tag="tnh")
                    for hc in range(4):
                        # pre[h, kl, q] = kproj[h, k0+kl] + qproj[h, q]
                        in0 = (
                            kproj[:, s, hc, k0 : k0 + Kg]
                            .unsqueeze(2)
                            .broadcast_to((128, Kg, Q))
                        )
                        in1 = (
                            qproj[:, s, hc, :]
                            .unsqueeze(1)
                            .broadcast_to((128, Kg, Q))
                        )
                        eng = nc.gpsimd if hc == 3 else nc.vector
                        eng.tensor_add(pre[:, hc, :Kg, :], in0, in1)
                    nc.scalar.activation(
                        tnh[:, :, :Kg, :], pre[:, :, :Kg, :], Tanh
                    )
                    for kl in range(Kg):
                        for hc in range(4):
                            nc.tensor.matmul(
                                psc[hc][:, k0 + kl : k0 + kl + 1],
                                tnh[:, hc, kl, :],
                                wv_sb[:, hc : hc + 1],
                                start=True,
                                stop=True,
                            )
                # ---- softmax over the V columns -------------------------
                msc = smp.tile([128, K], F32, tag="msc")
                nc.vector.tensor_add(msc[:, :V], psc[0][:, :V], mask_sb[:, s, :V])
                nc.vector.tensor_add(msc[:, :V], msc[:, :V], psc[1][:, :V])
                nc.vector.tensor_add(msc[:, :V], msc[:, :V], psc[2][:, :V])
                nc.vector.tensor_add(msc[:, :V], msc[:, :V], psc[3][:, :V])
                z = smp.tile([128, 1], F32, tag="z")
                nc.scalar.activation(
                    e_sb[:, s, :V], msc[:, :V], Exp, accum_out=z[:]
                )
                nc.vector.reciprocal(rz[:, s : s + 1], z[:])
                # ---- attn @ values --------------------------------------
                pt = ps_misc.tile([128, 128], BF16, tag="pt")
                nc.tensor.transpose(pt[:], e_sb[:, s, :], ident[:])
                eT = smp.tile([128, 128], BF16, tag="eT")
                nc.vector.tensor_copy(eT[:], pt[:])
                po = ps_misc.tile([128, D], F32, tag="po")
                nc.tensor.matmul(
                    po[:], eT[:V, :], vals_sb[:V, s, :], start=True, stop=True
                )
                o_sb = smp.tile([128, D], F32, tag="o")
                nc.vector.tensor_scalar_mul(o_sb[:], po[:], rz[:, s : s + 1])
                nc.sync.dma_start(out_d[s], o_sb[:])

    nc.finalize()
    return nc


def kernel(queries, keys, values, valid_lens, Wq, Wk, wv):
    queries = np.asarray(queries, dtype=np.float32)
    keys = np.asarray(keys, dtype=np.float32)
    values = np.asarray(values, dtype=np.float32)
    valid_lens = np.asarray(valid_lens, dtype=np.int32)
    Wq = np.asarray(Wq, dtype=np.float32)
    Wk = np.asarray(Wk, dtype=np.float32)
    wv = np.asarray(wv, dtype=np.float32)

    # ---- slot assignment: minimize V0 + V1 ------------------------------
    order = np.argsort(-valid_lens, kind="stable")
    slot0 = order[:NCORES]
    slot1 = order[NCORES:][::-1]  # pair largest with smallest
    V0 = int(valid_lens[slot0].max())
    V1 = int(valid_lens[slot1].max())

    key_ = (V0, V1)
    if key_ not in _NC_CACHE:
        _NC_CACHE[key_] = _build_nc(V0, V1)
    nc = _NC_CACHE[key_]

    # ---- host-side shard prep -------------------------------------------
    wq_bf = Wq.astype(BF16_NP)
    wk_bf = Wk.astype(BF16_NP)
    wv4 = np.ascontiguousarray(wv.reshape(4, 128).T).astype(BF16_NP)  # [128,4]

    in_maps = []
    for c in range(NCORES):
        bs = (int(slot0[c]), int(slot1[c]))
        qT = np.stack(
            [np.ascontiguousarray(queries[b].T) for b in bs]
        ).astype(BF16_NP)  # [2, D, Q]
        kTm = np.zeros((2, D, K), dtype=BF16_NP)
        valsm = np.zeros((2, K, D), dtype=BF16_NP)
        maskm = np.zeros((2, 128, K), dtype=np.float32)
        for s, b in enumerate(bs):
            v = int(valid_lens[b])
            kTm[s, :, :v] = keys[b, :v].T.astype(BF16_NP)
            valsm[s, :v] = values[b, :v].astype(BF16_NP)
            maskm[s, :, v:] = NEG
        in_maps.append(
            {
                "qT": qT,
                "kT": kTm,
                "vals": valsm,
                "wq": wq_bf,
                "wk": wk_bf,
                "wv4": wv4,
                "mask": maskm,
            }
        )

    res = run_bass_kernel_spmd(nc, in_maps, list(range(NCORES)))
    global LAST_RESULT
    LAST_RESULT = res

    out = np.empty((B, Q, D), dtype=np.float32)
    for c in range(NCORES):
        o = np.asarray(res.results[c]["out"], dtype=np.float32)
        out[int(slot0[c])] = o[0]
        out[int(slot1[c])] = o[1]
    return out


# revision 10
# speedup vs baseline: 1.1288x; 1.1084x over previous
"""Trainium2 Bass kernel for nn_AdditiveAttention (Bahdanau attention).

Reference computation (B=16, Q=128, K=128, D=512, H=512):
    q = queries @ Wq                     [B,Q,H]
    k = keys @ Wk                        [B,K,H]
    scores[b,q,k] = sum_h wv[h] * tanh(q[b,q,h] + k[b,k,h])
    attn = softmax over valid keys (k < valid_lens[b])
    out = attn @ values                  [B,Q,D]

Strategy (8 NeuronCores, data-parallel over batch):
  - 2 batches per core, in two "slots".  Slot sizes V0/V1 = max valid_len
    over the cores' slot-0/slot-1 batches; batches are assigned to slots
    sorted by valid_len so V0+V1 is minimal.  Columns >= the batch's own
    valid_len are masked with a -1e9 additive mask (softmax -> 0).
  - On-device per slot: project queriesT/keysT transposed ([h=partitions]),
    per key-column k: pre[h,q] = q_projT + k_col (DVE tensor_scalar, 4x
    mode), tanh on ScalarE in big batched instructions, then the wv
    reduction on TensorE (lhsT = tanh tile [128h,128q], rhs = wv chunk
    [128h,1] -> scores column per h-chunk into 4 per-chunk PSUM banks),
    bank-sum + mask on DVE, exp with fused accumulated sum on ScalarE,
    reciprocal on DVE, transpose of the exp matrix on TensorE, and a final
    attn @ values matmul, scaled by 1/z.
All heavy compute is bf16 on PE/DVE with fp32 PSUM accumulation; tanh/exp
run in fp32 internally on ScalarE.  Masked key columns are skipped
entirely (only V0+V1 of 256 columns are computed).
"""

import os
import sys
import types
import math
import numpy as np
import ml_dtypes

# ---------------------------------------------------------------------------
# axon NTFF profile hook (lets trace=True / BASS_TRACE=1 work in this image)
# ---------------------------------------------------------------------------
def _install_axon_hooks():
    if "antenv.axon_hooks" in sys.modules:
        return
    try:
        import trn_agent_boot.trn_boot as _tb

        _hooks = types.ModuleType("antenv.axon_hooks")
        _hook = _tb._ntff_profile_via_ctypes("/opt/axon/libaxon_pjrt.so")
        _hooks.get_axon_ntff_profile_hook = lambda: _hook
        _hooks.set_axon_ntff_profile_hook = lambda h: None
        sys.modules["antenv.axon_hooks"] = _hooks
    except Exception:
        pass


_install_axon_hooks()

import concourse.bass as bass
import concourse.bacc as bacc
import concourse.mybir as mybir
import concourse.tile as tile
import concourse.bass_utils as bass_utils
from concourse.bass_utils import run_bass_kernel_spmd
from concourse.masks import make_identity

# Avoid S3 artifact-upload attempts in the trace path.
bass_utils.upload_artifacts = lambda tmpdir: tmpdir

F32 = mybir.dt.float32
BF16 = mybir.dt.bfloat16
BF16_NP = ml_dtypes.bfloat16

B, Q, K, D, H = 16, 128, 128, 512, 512
NCORES = 8
KT = 32  # key-columns per tanh group
NEG = -1e9

_NC_CACHE: dict = {}


def _build_nc(V0: int, V1: int):
    """Build + finalize the single-core SPMD program for slot sizes V0, V1."""
    nc = bacc.Bacc(None, target_bir_lowering=False, debug=False)

    qT = nc.declare_dram_parameter("qT", [2, D, Q], BF16, isOutput=False)
    kT = nc.declare_dram_parameter("kT", [2, D, K], BF16, isOutput=False)
    vals = nc.declare_dram_parameter("vals", [2, K, D], BF16, isOutput=False)
    wq_d = nc.declare_dram_parameter("wq", [D, H], BF16, isOutput=False)
    wk_d = nc.declare_dram_parameter("wk", [D, H], BF16, isOutput=False)
    wv_d = nc.declare_dram_parameter("wv4", [128, 4], BF16, isOutput=False)
    mask_d = nc.declare_dram_parameter("mask", [2, 128, K], F32, isOutput=False)
    out_d = nc.declare_dram_parameter("out", [2, Q, D], F32, isOutput=True)

    Vs = [V0, V1]
    Tanh = mybir.ActivationFunctionType.Tanh
    Exp = mybir.ActivationFunctionType.Exp

    with tile.TileContext(nc) as tc:
        with (
            tc.tile_pool(name="const", bufs=1) as constp,
            tc.tile_pool(name="io", bufs=1) as iop,
            tc.tile_pool(name="proj", bufs=1) as projp,
            tc.tile_pool(name="stage", bufs=2) as stagep,
            tc.tile_pool(name="sm", bufs=2) as smp,
            tc.tile_pool(name="ps_proj", bufs=2, space="PSUM") as ps_proj,
            tc.tile_pool(name="ps_sc", bufs=4, space="PSUM") as ps_sc,
            tc.tile_pool(name="ps_misc", bufs=1, space="PSUM") as ps_misc,
        ):
            # ---- constants & inputs -------------------------------------
            wq_sb = constp.tile([128, 4, H], BF16, tag="wq")
            nc.sync.dma_start(wq_sb[:], wq_d[:].rearrange("(c p) h -> p c h", p=128))
            wk_sb = constp.tile([128, 4, H], BF16, tag="wk")
            nc.sync.dma_start(wk_sb[:], wk_d[:].rearrange("(c p) h -> p c h", p=128))
            wv_sb = constp.tile([128, 4], BF16, tag="wv")
            nc.sync.dma_start(wv_sb[:], wv_d[:])
            ident = constp.tile([128, 128], BF16, tag="ident")
            make_identity(nc, ident[:])

            qt_sb = iop.tile([128, 2, 4, Q], BF16, tag="qt")
            nc.sync.dma_start(qt_sb[:], qT[:].rearrange("s (c p) q -> p s c q", p=128))
            kt_sb = iop.tile([128, 2, 4, K], BF16, tag="kt")
            nc.sync.dma_start(kt_sb[:], kT[:].rearrange("s (c p) k -> p s c k", p=128))
            vals_sb = iop.tile([128, 2, D], BF16, tag="vals")
            nc.sync.dma_start(vals_sb[:], vals[:].rearrange("s k d -> k s d"))
            mask_sb = iop.tile([128, 2, K], F32, tag="mask")
            nc.sync.dma_start(mask_sb[:], mask_d[:].rearrange("s p k -> p s k"))

            # ---- projections: projT[h,q] = sum_d W[d,h] * xT[d,q] -------
            qproj = projp.tile([128, 2, 4, Q], BF16, tag="qproj")
            kproj = projp.tile([128, 2, 4, K], BF16, tag="kproj")
            for s in (0, 1):
                V = Vs[s]
                for hc in range(4):
                    pq = ps_proj.tile([128, 128], F32, tag="pp")
                    for dc in range(4):
                        nc.tensor.matmul(
                            pq[:],
                            wq_sb[:, dc, hc * 128 : (hc + 1) * 128],
                            qt_sb[:, s, dc, :],
                            start=(dc == 0),
                            stop=(dc == 3),
                        )
                    nc.scalar.copy(qproj[:, s, hc, :], pq[:])
                    pk = ps_proj.tile([128, 128], F32, tag="pp")
                    for dc in range(4):
                        nc.tensor.matmul(
                            pk[:, :V],
                            wk_sb[:, dc, hc * 128 : (hc + 1) * 128],
                            kt_sb[:, s, dc, :V],
                            start=(dc == 0),
                            stop=(dc == 3),
                        )
                    nc.scalar.copy(kproj[:, s, hc, :V], pk[:, :V])

            # persistent softmax state
            e_sb = projp.tile([128, 2, K], BF16, tag="e")
            nc.vector.memset(e_sb[:], 0.0)
            rz = projp.tile([128, 2], F32, tag="rz")

            # ---- main loop ----------------------------------------------
            for s in (0, 1):
                V = Vs[s]
                ngroups = math.ceil(V / KT)
                # 4 per-h-chunk score banks, each column written exactly once
                psc = [
                    ps_sc.tile([128, K], F32, tag="psc", name=f"psc{s}_{i}")
                    for i in range(4)
                ]
                for g in range(ngroups):
                    k0 = g * KT
                    Kg = min(KT, V - k0)
                    pre = stagep.tile([128, 4, KT, Q], BF16, tag="pre")
                    tnh = stagep.tile([128, 4, KT, Q], BF16, tag="tnh")
                    for hc in range(4):
                        # pre[h, kl, q] = kproj[h, k0+kl] + qproj[h, q]
                        in0 = (
                            kproj[:, s, hc, k0 : k0 + Kg]
                            .unsqueeze(2)
                            .broadcast_to((128, Kg, Q))
                        )
                        in1 = (
                            qproj[:, s, hc, :]
                            .unsqueeze(1)
                            .broadcast_to((128, Kg, Q))
                        )
                        nc.vector.tensor_add(pre[:, hc, :Kg, :], in0, in1)
                    nc.scalar.activation(
                        tnh[:, :, :Kg, :], pre[:, :, :Kg, :], Tanh
                    )
                    for kl in range(Kg):
                        for hc in range(4):
                            nc.tensor.matmul(
                                psc[hc][:, k0 + kl : k0 + kl + 1],
                                tnh[:, hc, kl, :],
                                wv_sb[:, hc : hc + 1],
                                start=True,
                                stop=True,
                            )
                # ---- softmax over the V columns -------------------------
                msc = smp.tile([128, K], F32, tag="msc")
                nc.vector.tensor_add(msc[:, :V], psc[0][:, :V], mask_sb[:, s, :V])
                nc.vector.tensor_add(msc[:, :V], msc[:, :V], psc[1][:, :V])
                nc.vector.tensor_add(msc[:, :V], msc[:, :V], psc[2][:, :V])
                nc.vector.tensor_add(msc[:, :V], msc[:, :V], psc[3][:, :V])
                z = smp.tile([128, 1], F32, tag="z")
                nc.scalar.activation(
                    e_sb[:, s, :V], msc[:, :V], Exp, accum_out=z[:]
                )
                nc.vector.reciprocal(rz[:, s : s + 1], z[:])
                # ---- attn @ values --------------------------------------
                pt = ps_misc.tile([128, 128], BF16, tag="pt")
                nc.tensor.transpose(pt[:], e_sb[:, s, :], ident[:])
                eT = smp.tile([128, 128], BF16, tag="eT")
                nc.vector.tensor_copy(eT[:], pt[:])
                po = ps_misc.tile([128, D], F32, tag="po")
                nc.tensor.matmul(
                    po[:], eT[:V, :], vals_sb[:V, s, :], start=True, stop=True
                )
                o_sb = smp.tile([128, D], F32, tag="o")
                nc.vector.tensor_scalar_mul(o_sb[:], po[:], rz[:, s : s + 1])
                nc.sync.dma_start(out_d[s], o_sb[:])

    nc.finalize()
    return nc


def kernel(queries, keys, values, valid_lens, Wq, Wk, wv):
    queries = np.asarray(queries, dtype=np.float32)
    keys = np.asarray(keys, dtype=np.float32)
    values = np.asarray(values, dtype=np.float32)
    valid_lens = np.asarray(valid_lens, dtype=np.int32)
    Wq = np.asarray(Wq, dtype=np.float32)
    Wk = np.asarray(Wk, dtype=np.float32)
    wv = np.asarray(wv, dtype=np.float32)

    # ---- slot assignment: minimize V0 + V1 ------------------------------
    order = np.argsort(-valid_lens, kind="stable")
    slot0 = order[:NCORES]
    slot1 = order[NCORES:][::-1]  # pair largest with smallest
    V0 = int(valid_lens[slot0].max())
    V1 = int(valid_lens[slot1].max())

    key_ = (V0, V1)
    if key_ not in _NC_CACHE:
        _NC_CACHE[key_] = _build_nc(V0, V1)
    nc = _NC_CACHE[key_]

    # ---- host-side shard prep -------------------------------------------
    wq_bf = Wq.astype(BF16_NP)
    wk_bf = Wk.astype(BF16_NP)
    wv4 = np.ascontiguousarray(wv.reshape(4, 128).T).astype(BF16_NP)  # [128,4]

    in_maps = []
    for c in range(NCORES):
        bs = (int(slot0[c]), int(slot1[c]))
        qT = np.stack(
            [np.ascontiguousarray(queries[b].T) for b in bs]
        ).astype(BF16_NP)  # [2, D, Q]
        kTm = np.zeros((2, D, K), dtype=BF16_NP)
        valsm = np.zeros((2, K, D), dtype=BF16_NP)
        maskm = np.zeros((2, 128, K), dtype=np.float32)
        for s, b in enumerate(bs):
            v = int(valid_lens[b])
            kTm[s, :, :v] = keys[b, :v].T.astype(BF16_NP)
            valsm[s, :v] = values[b, :v].astype(BF16_NP)
            maskm[s, :, v:] = NEG
        in_maps.append(
            {
                "qT": qT,
                "kT": kTm,
                "vals": valsm,
                "wq": wq_bf,
                "wk": wk_bf,
                "wv4": wv4,
                "mask": maskm,
            }
        )

    res = run_bass_kernel_spmd(nc, in_maps, list(range(NCORES)))
    global LAST_RESULT
    LAST_RESULT = res

    out = np.empty((B, Q, D), dtype=np.float32)
    for c in range(NCORES):
        o = np.asarray(res.results[c]["out"], dtype=np.float32)
        out[int(slot0[c])] = o[0]
        out[int(slot1[c])] = o[1]
    return out


# revision 13
# speedup vs baseline: 1.1390x; 1.0090x over previous
"""Trainium2 Bass kernel for nn_AdditiveAttention (Bahdanau attention).

Reference computation (B=16, Q=128, K=128, D=512, H=512):
    q = queries @ Wq                     [B,Q,H]
    k = keys @ Wk                        [B,K,H]
    scores[b,q,k] = sum_h wv[h] * tanh(q[b,q,h] + k[b,k,h])
    attn = softmax over valid keys (k < valid_lens[b])
    out = attn @ values                  [B,Q,D]

Strategy (8 NeuronCores, SPMD data-parallel):
  Work is proportional to (#queries x valid_len) per batch, and queries are
  embarrassingly parallel (each query's softmax is independent).  Each batch
  is split into q-range fragments; fragments are sorted by valid_len and
  distributed over 8 cores x S uniform "slots" (one SPMD program).  Slot j
  has a fixed shape (Qs_j, V_j) = (fragment q-size, max valid_len in that
  slot across cores); shorter fragments are handled with an additive -1e9
  key mask.  Sorted assignment keeps slot V_j close to each member's
  valid_len, minimizing padded work.

  On-device per slot: project queriesT/keysT transposed ([h=partitions]),
  per key-column k: pre[h,q] = q_projT + k_col broadcast-add on DVE (grouped
  stride-0 tensor_tensor), tanh on ScalarE in big batched instructions, wv
  reduction on TensorE (lhsT = tanh tile [128h,Qs], rhs = wv chunk [128h,1]
  -> scores column, one independent matmul per (k, h-chunk) into 4
  per-chunk PSUM banks), bank-sum + mask on DVE, exp with fused accumulated
  sum on ScalarE, reciprocal on DVE, transpose of the exp matrix on
  TensorE, and a final attn @ values matmul scaled by 1/z.
  bf16 on PE/DVE with fp32 PSUM accumulation; tanh/exp are fp32 internally.
"""

import os
import sys
import types
import math
import numpy as np
import ml_dtypes

# ---------------------------------------------------------------------------
# axon NTFF profile hook (lets trace=True / BASS_TRACE=1 work in this image)
# ---------------------------------------------------------------------------
def _install_axon_hooks():
    if "antenv.axon_hooks" in sys.modules:
        return
    try:
        import trn_agent_boot.trn_boot as _tb

        _hooks = types.ModuleType("antenv.axon_hooks")
        _hook = _tb._ntff_profile_via_ctypes("/opt/axon/libaxon_pjrt.so")
        _hooks.get_axon_ntff_profile_hook = lambda: _hook
        _hooks.set_axon_ntff_profile_hook = lambda h: None
        sys.modules["antenv.axon_hooks"] = _hooks
    except Exception:
        pass


_install_axon_hooks()

import concourse.bass as bass
import concourse.bacc as bacc
import concourse.mybir as mybir
import concourse.tile as tile
import concourse.bass_utils as bass_utils
from concourse.bass_utils import run_bass_kernel_spmd
from concourse.masks import make_identity

# Avoid S3 artifact-upload attempts in the trace path.
bass_utils.upload_artifacts = lambda tmpdir: tmpdir

F32 = mybir.dt.float32
BF16 = mybir.dt.bfloat16
BF16_NP = ml_dtypes.bfloat16

B, Q, K, D, H = 16, 128, 128, 512, 512
NCORES = 8
KT = 32  # key-columns per tanh group
NEG = -1e9

_NC_CACHE: dict = {}
LAST_RESULT = None


def _plan(valid_lens):
    """Pick fragmentation scheme; return (slots, content).

    slots:   tuple of (Qs, V) uniform slot shapes
    content: per-core list of fragments (batch, q0, qs, v), one per slot
    """
    best = None
    for nsplit in (1, 2):
        S = (B * nsplit) // NCORES
        qs = Q // nsplit
        frags = [
            (b, i * qs, qs, int(valid_lens[b]))
            for b in range(B)
            for i in range(nsplit)
        ]
        frags.sort(key=lambda f: -f[3])
        slots = []
        content = [[] for _ in range(NCORES)]
        cost = 0.0
        for j in range(S):
            grp = frags[j * NCORES : (j + 1) * NCORES]
            V = max(f[3] for f in grp)
            slots.append((qs, V))
            cost += qs * V
            for c, f in enumerate(grp):
                content[c].append(f)
        cost += S * 600.0  # per-slot fixed overhead, in q*k units
        if best is None or cost < best[0]:
            best = (cost, tuple(slots), content)
    return best[1], best[2]


def _build_nc(slots):
    """Build + finalize the single-core SPMD program for the given slots."""
    S = len(slots)
    maxQs = max(q for q, _ in slots)
    nc = bacc.Bacc(None, target_bir_lowering=False, debug=False)

    qT = nc.declare_dram_parameter("qT", [S, D, 128], BF16, isOutput=False)
    kT = nc.declare_dram_parameter("kT", [S, D, 128], BF16, isOutput=False)
    vals = nc.declare_dram_parameter("vals", [S, K, D], BF16, isOutput=False)
    wq_d = nc.declare_dram_parameter("wq", [D, H], BF16, isOutput=False)
    wk_d = nc.declare_dram_parameter("wk", [D, H], BF16, isOutput=False)
    wv_d = nc.declare_dram_parameter("wv4", [128, 4], BF16, isOutput=False)
    mask_d = nc.declare_dram_parameter("mask", [S, 128, K], F32, isOutput=False)
    out_d = nc.declare_dram_parameter("out", [S, 128, D], F32, isOutput=True)

    Tanh = mybir.ActivationFunctionType.Tanh
    Exp = mybir.ActivationFunctionType.Exp
    stage_bufs = 3 if maxQs <= 64 else 2

    with tile.TileContext(nc) as tc:
        with (
            tc.tile_pool(name="const", bufs=1) as constp,
            tc.tile_pool(name="io", bufs=1) as iop,
            tc.tile_pool(name="proj", bufs=1) as projp,
            tc.tile_pool(name="stage", bufs=stage_bufs) as stagep,
            tc.tile_pool(name="sm", bufs=2) as smp,
            tc.tile_pool(name="ps_proj", bufs=2, space="PSUM") as ps_proj,
            tc.tile_pool(name="ps_sc", bufs=4, space="PSUM") as ps_sc,
            tc.tile_pool(name="ps_misc", bufs=1, space="PSUM") as ps_misc,
        ):
            # ---- constants & inputs -------------------------------------
            wq_sb = constp.tile([128, 4, H], BF16, tag="wq")
            nc.sync.dma_start(wq_sb[:], wq_d[:].rearrange("(c p) h -> p c h", p=128))
            wk_sb = constp.tile([128, 4, H], BF16, tag="wk")
            nc.sync.dma_start(wk_sb[:], wk_d[:].rearrange("(c p) h -> p c h", p=128))
            wv_sb = constp.tile([128, 4], BF16, tag="wv")
            nc.sync.dma_start(wv_sb[:], wv_d[:])
            ident = constp.tile([128, 128], BF16, tag="ident")
            make_identity(nc, ident[:])

            qt_sb = iop.tile([128, S, 4, 128], BF16, tag="qt")
            nc.sync.dma_start(qt_sb[:], qT[:].rearrange("s (c p) q -> p s c q", p=128))
            kt_sb = iop.tile([128, S, 4, 128], BF16, tag="kt")
            nc.sync.dma_start(kt_sb[:], kT[:].rearrange("s (c p) k -> p s c k", p=128))
            vals_sb = iop.tile([128, S, D], BF16, tag="vals")
            nc.sync.dma_start(vals_sb[:], vals[:].rearrange("s k d -> k s d"))
            mask_sb = iop.tile([128, S, K], F32, tag="mask")
            nc.sync.dma_start(mask_sb[:], mask_d[:].rearrange("s p k -> p s k"))

            # ---- projections: projT[h,x] = sum_d W[d,h] * xT[d,x] -------
            qproj = projp.tile([128, S, 4, 128], BF16, tag="qproj")
            kproj = projp.tile([128, S, 4, 128], BF16, tag="kproj")
            for s in range(S):
                Qs, V = slots[s]
                for hc in range(4):
                    pq = ps_proj.tile([128, 128], F32, tag="pp")
                    for dc in range(4):
                        nc.tensor.matmul(
                            pq[:, :Qs],
                            wq_sb[:, dc, hc * 128 : (hc + 1) * 128],
                            qt_sb[:, s, dc, :Qs],
                            start=(dc == 0),
                            stop=(dc == 3),
                        )
                    nc.scalar.copy(qproj[:, s, hc, :Qs], pq[:, :Qs])
                    pk = ps_proj.tile([128, 128], F32, tag="pp")
                    for dc in range(4):
                        nc.tensor.matmul(
                            pk[:, :V],
                            wk_sb[:, dc, hc * 128 : (hc + 1) * 128],
                            kt_sb[:, s, dc, :V],
                            start=(dc == 0),
                            stop=(dc == 3),
                        )
                    nc.scalar.copy(kproj[:, s, hc, :V], pk[:, :V])

            # persistent softmax state
            e_sb = projp.tile([128, S, 128], BF16, tag="e")
            nc.vector.memset(e_sb[:], 0.0)
            rz = projp.tile([128, S], F32, tag="rz")

            # ---- main loop ----------------------------------------------
            for s in range(S):
                Qs, V = slots[s]
                ngroups = math.ceil(V / KT)
                # 4 per-h-chunk score banks, each column written exactly once
                psc = [
                    ps_sc.tile([128, K], F32, tag="psc", name=f"psc{s}_{i}")
                    for i in range(4)
                ]
                for g in range(ngroups):
                    k0 = g * KT
                    Kg = min(KT, V - k0)
                    pre = stagep.tile([128, 4, KT, maxQs], BF16, tag="pre")
                    tnh = stagep.tile([128, 4, KT, maxQs], BF16, tag="tnh")
                    for hc in range(4):
                        # pre[h, kl, q] = kproj[h, k0+kl] + qproj[h, q]
                        in0 = (
                            kproj[:, s, hc, k0 : k0 + Kg]
                            .unsqueeze(2)
                            .broadcast_to((128, Kg, Qs))
                        )
                        in1 = (
                            qproj[:, s, hc, :Qs]
                            .unsqueeze(1)
                            .broadcast_to((128, Kg, Qs))
                        )
                        nc.vector.tensor_add(pre[:, hc, :Kg, :Qs], in0, in1)
                    nc.scalar.activation(
                        tnh[:, :, :Kg, :Qs], pre[:, :, :Kg, :Qs], Tanh
                    )
                    for kl in range(Kg):
                        for hc in range(4):
                            nc.tensor.matmul(
                                psc[hc][:Qs, k0 + kl : k0 + kl + 1],
                                tnh[:, hc, kl, :Qs],
                                wv_sb[:, hc : hc + 1],
                                start=True,
                                stop=True,
                            )
                # ---- softmax over the V columns -------------------------
                msc = smp.tile([128, K], F32, tag="msc")
                nc.vector.tensor_add(
                    msc[:Qs, :V], psc[0][:Qs, :V], mask_sb[:Qs, s, :V]
                )
                nc.vector.tensor_add(msc[:Qs, :V], msc[:Qs, :V], psc[1][:Qs, :V])
                nc.vector.tensor_add(msc[:Qs, :V], msc[:Qs, :V], psc[2][:Qs, :V])
                nc.vector.tensor_add(msc[:Qs, :V], msc[:Qs, :V], psc[3][:Qs, :V])
                z = smp.tile([128, 1], F32, tag="z")
                nc.scalar.activation(
                    e_sb[:Qs, s, :V], msc[:Qs, :V], Exp, accum_out=z[:Qs, :]
                )
                nc.vector.reciprocal(rz[:Qs, s : s + 1], z[:Qs, :])
                # ---- attn @ values --------------------------------------
                pt = ps_misc.tile([128, 128], BF16, tag="pt")
                nc.tensor.transpose(pt[:], e_sb[:, s, :], ident[:])
                eT = smp.tile([128, 128], BF16, tag="eT")
                nc.vector.tensor_copy(eT[:], pt[:])
                po = ps_misc.tile([128, D], F32, tag="po")
                nc.tensor.matmul(
                    po[:, :], eT[:V, :], vals_sb[:V, s, :], start=True, stop=True
                )
                o_sb = smp.tile([128, D], F32, tag="o")
                nc.vector.tensor_scalar_mul(
                    o_sb[:Qs, :], po[:Qs, :], rz[:Qs, s : s + 1]
                )
                nc.sync.dma_start(out_d[s, :Qs], o_sb[:Qs, :])

    nc.finalize()
    return nc


def kernel(queries, keys, values, valid_lens, Wq, Wk, wv):
    global LAST_RESULT
    queries = np.asarray(queries, dtype=np.float32)
    keys = np.asarray(keys, dtype=np.float32)
    values = np.asarray(values, dtype=np.float32)
    valid_lens = np.asarray(valid_lens, dtype=np.int32)
    Wq = np.asarray(Wq, dtype=np.float32)
    Wk = np.asarray(Wk, dtype=np.float32)
    wv = np.asarray(wv, dtype=np.float32)

    slots, content = _plan(valid_lens)
    S = len(slots)

    if slots not in _NC_CACHE:
        _NC_CACHE[slots] = _build_nc(slots)
    nc = _NC_CACHE[slots]

    # ---- host-side shard prep -------------------------------------------
    wq_bf = Wq.astype(BF16_NP)
    wk_bf = Wk.astype(BF16_NP)
    wv4 = np.ascontiguousarray(wv.reshape(4, 128).T).astype(BF16_NP)  # [128,4]

    in_maps = []
    for c in range(NCORES):
        qTm = np.zeros((S, D, 128), dtype=BF16_NP)
        kTm = np.zeros((S, D, 128), dtype=BF16_NP)
        valsm = np.zeros((S, K, D), dtype=BF16_NP)
        maskm = np.zeros((S, 128, K), dtype=np.float32)
        for s, (b, q0, qs, v) in enumerate(content[c]):
            qTm[s, :, :qs] = queries[b, q0 : q0 + qs].T.astype(BF16_NP)
            kTm[s, :, :v] = keys[b, :v].T.astype(BF16_NP)
            valsm[s, :v] = values[b, :v].astype(BF16_NP)
            maskm[s, :, v:] = NEG
        in_maps.append(
            {
                "qT": qTm,
                "kT": kTm,
                "vals": valsm,
                "wq": wq_bf,
                "wk": wk_bf,
                "wv4": wv4,
                "mask": maskm,
            }
        )

    res = run_bass_kernel_spmd(nc, in_maps, list(range(NCORES)))
    LAST_RESULT = res

    out = np.empty((B, Q, D), dtype=np.float32)
    for c in range(NCORES):
        o = np.asarray(res.results[c]["out"], dtype=np.float32)
        for s, (b, q0, qs, v) in enumerate(content[c]):
            out[b, q0 : q0 + qs] = o[s, :qs]
    return out
